# revision 11
# baseline (speedup 1.0000x reference)
"""Trainium2 Bass kernel for nn_Block_15144054685914 (dense transformer block).

Sharding: 8 cores = 2 batch groups (DP) x 4-way tensor parallel.
  core c: batch b = c//4, heads [4*(c%4), 4*(c%4)+4), FFN slice c%4.
One on-device bf16 AllReduce per t-chunk (attention residual) within each
4-core batch group; final partial outputs summed on host.

Math tricks (all exact up to float rounding):
  - rmsnorm(x) scale cancels for Q/K (rmsnorm(rope(c*v)) == rmsnorm(rope(v)))
  - rmsnorm scale for the MLP folds into a per-row s2^2 post-scale
  - softmax 1/sum folds into a post-PV column scale; row sums via ones-matmul
  - alpha softmax mixing + ve mixing folded into weights on host
  - Q/K/V and MLP-fc GEMMs run as split-fp8 (hi+lo e4m3) DoubleRow matmuls:
    x*s = hi + lo exactly-scaled; x@W ~ xh@Wh + xl@Wh + xh@Wl accumulated in
    one PSUM group at 0.5 cycles/row (25% faster than bf16, ~2x more accurate)
  - partition_all_reduce/partition_broadcast (Pool engine) replace the
    ones-matmul reduce/broadcast tricks for q/k norm and 1/sum broadcast
"""

import math
import numpy as np
import ml_dtypes

B, E, H, J = 2, 2048, 16, 4
D = 128
GC = 12
FF = 4 * E
NCORES = 8
HL = H // 4            # local heads per core
HDL = HL * D           # 512
FL = FF // 4           # 2048 local ffn rows
EPS = float(np.finfo(np.float32).eps)
T_FULL = 2048
CH = 512               # t-chunk for attention + AllReduce
EC = E // 128          # 16
FCT = FL // 128        # 16 f-tiles

SX = 16.0              # fp8 scale for x / x1 activations
SW = 256.0             # fp8 scale for Wq/Wk/Wv/Wfc weights
SG = 16.0              # fp8 scale for Wg
PSC = 1.0 / (SX * SW)  # 2^-12 PSUM descale

bf16n = ml_dtypes.bfloat16
f8n = ml_dtypes.float8_e4m3
NOAR = False


def _bf(x):
    return np.ascontiguousarray(np.asarray(x, dtype=np.float32)).astype(bf16n)


def _split8(x, s):
    xs = np.ascontiguousarray(np.asarray(x, np.float32)) * np.float32(s)
    hi = xs.astype(f8n)
    lo = (xs - hi.astype(np.float32)).astype(f8n)
    return hi, lo


def _softmax32(a):
    a = np.asarray(a, np.float32)
    e = np.exp(a - a.max(-1, keepdims=True))
    return (e / e.sum(-1, keepdims=True)).astype(np.float32)


def shard_inputs(x, ve, cos, sin, Wq, Wk, Wv, Wo, alpha_k, alpha_v, Wg,
                 Wfc, Wmlp, T=T_FULL):
    x = np.asarray(x, np.float32)[:, :T]
    ve = np.asarray(ve, np.float32)[:, :T]
    cosf = np.asarray(cos, np.float32)[0, :T, 0, :]   # (T, 64)
    sinf = np.asarray(sin, np.float32)[0, :T, 0, :]
    Wq = np.asarray(Wq, np.float32)
    Wk = np.asarray(Wk, np.float32)
    Wv = np.asarray(Wv, np.float32)
    Wo = np.asarray(Wo, np.float32)
    Wg = np.asarray(Wg, np.float32)
    Wfc = np.asarray(Wfc, np.float32)
    Wmlp = np.asarray(Wmlp, np.float32)

    wk_s = _softmax32(alpha_k)            # (H, J)
    wv_s = _softmax32(alpha_v)
    Wk_eff = np.einsum('hj,jde->hde', wk_s,
                       Wk.reshape(J, D, E)).reshape(H * D, E)
    Wv_eff = np.einsum('hj,jde->hde', wv_s,
                       Wv.reshape(J, D, E)).reshape(H * D, E)

    # causal 0/1 masks for the 4 s-blocks crossing the diagonal of a chunk
    masks = np.zeros((4, 128, CH), np.float32)
    for m in range(4):
        s_idx = np.arange(128)[:, None] + m * 128
        t_idx = np.arange(CH)[None, :]
        masks[m] = (s_idx <= t_idx).astype(np.float32)

    in_maps = []
    for c in range(NCORES):
        b = c // 4
        hg = c % 4
        hsl = slice(hg * HDL, (hg + 1) * HDL)      # head-dim slice
        fsl = slice(hg * FL, (hg + 1) * FL)        # ffn slice
        hh = slice(hg * HL, (hg + 1) * HL)         # head slice
        xthi, xtlo = _split8(x[b].T, SX)           # (E, T)
        wqh, wql = _split8(Wq[hsl, :].T, SW)       # (E, HDL)
        wkh, wkl = _split8(Wk_eff[hsl, :].T, SW)
        wvh, wvl = _split8(Wv_eff[hsl, :].T, SW)
        wfh, wfl = _split8(Wfc[fsl, :].T, SW)      # (E, FL)
        wgh, wgl = _split8(Wg[hh, :].T, SG)        # (GC, HL)
        ve_m = np.einsum('hj,tjd->thd', wv_s[hh],
                         ve[b].reshape(T, J, D)).reshape(T, HDL)
        m = {
            "xq": _bf(0.25 * x[b]),                            # (T, E)
            "xthi": xthi, "xtlo": xtlo,
            "cos2": _bf(np.concatenate([cosf.T, cosf.T], 0)),  # (128, T)
            "sin2": _bf(np.concatenate([sinf.T, -sinf.T], 0)),
            "p64": _bf(np.eye(128)[:, list(range(64, 128)) + list(range(64))].T),
            "wqh": wqh, "wql": wql, "wkh": wkh, "wkl": wkl,
            "wvh": wvh, "wvl": wvl, "wfh": wfh, "wfl": wfl,
            "wgh": wgh, "wgl": wgl,
            "woT": _bf(Wo.T[hsl, :]),                          # (HDL, E)
            "wmlpT": _bf(Wmlp.T[fsl, :]),                      # (FL, E)
            "vem": _bf(ve_m),                                  # (T, HDL)
            "masks": _bf(masks),                               # (4, 128, CH)
            "ident": _bf(np.eye(128)),
            "onec": _bf(np.ones((128, 1))),
        }
        in_maps.append(m)
    return in_maps


def declare_io(nc, T):
    import concourse.mybir as mybir
    bf = mybir.dt.bfloat16
    f8 = mybir.dt.float8e4
    io = {}

    def inp(name, shape, dt=bf):
        io[name] = nc.dram_tensor(name, list(shape), dt,
                                  kind="ExternalInput").ap()

    inp("xq", (T, E))
    inp("xthi", (E, T), f8); inp("xtlo", (E, T), f8)
    inp("cos2", (128, T)); inp("sin2", (128, T)); inp("p64", (128, 128))
    inp("wqh", (E, HDL), f8); inp("wql", (E, HDL), f8)
    inp("wkh", (E, HDL), f8); inp("wkl", (E, HDL), f8)
    inp("wvh", (E, HDL), f8); inp("wvl", (E, HDL), f8)
    inp("wfh", (E, FL), f8); inp("wfl", (E, FL), f8)
    inp("wgh", (GC, HL), f8); inp("wgl", (GC, HL), f8)
    inp("woT", (HDL, E)); inp("wmlpT", (FL, E))
    inp("vem", (T, HDL))
    inp("masks", (4, 128, CH)); inp("ident", (128, 128))
    inp("onec", (128, 1))
    io["out"] = nc.dram_tensor("out", [T, E], bf, kind="ExternalOutput").ap()
    io["out_x1"] = nc.dram_tensor("out_x1", [T, E], bf,
                                  kind="ExternalOutput").ap()
    return io


def emit(tc, io, T):
    import concourse.bass_isa as bass_isa
    import concourse.mybir as mybir
    from contextlib import ExitStack

    nc = tc.nc
    bf = mybir.dt.bfloat16
    f8 = mybir.dt.float8e4
    f32 = mybir.dt.float32
    AF = mybir.ActivationFunctionType
    OP = mybir.AluOpType
    DR = mybir.MatmulPerfMode.DoubleRow
    RED = bass_isa.ReduceOp
    nch = T // CH
    qk_ln_scale = 1.0 / (128.0 * 1.44)
    # q/k psums carry a SX*SW scale; ln bias must carry its square
    eps_qk_v = (SX * SW) ** 2 * EPS / 1.44
    inv_sqrt_d = 1.0 / math.sqrt(D)

    with ExitStack() as ctx:
        cpool = ctx.enter_context(tc.tile_pool(name="const", bufs=1))
        big = ctx.enter_context(tc.tile_pool(name="big", bufs=1))
        wk = ctx.enter_context(tc.tile_pool(name="wk", bufs=1))
        colp = ctx.enter_context(tc.tile_pool(name="colp", bufs=1))
        psp = ctx.enter_context(tc.tile_pool(name="psp", bufs=1, space="PSUM"))
        dram = ctx.enter_context(tc.tile_pool(name="dram", bufs=2,
                                              space="DRAM"))

        # ---------------- chunk-0 stream prefetch ----------------
        xt_tiles = {}

        def load_xt(ci):
            th = big.tile([128, EC, CH], f8, name=f"xth{ci}", tag="xth",
                          bufs=2)
            nc.sync.dma_start(
                th[:], io["xthi"].rearrange("(a p) t -> p a t", p=128)
                [:, :, ci * CH:(ci + 1) * CH])
            tl = big.tile([128, EC, CH], f8, name=f"xtl{ci}", tag="xtl",
                          bufs=1)
            nc.sync.dma_start(
                tl[:], io["xtlo"].rearrange("(a p) t -> p a t", p=128)
                [:, :, ci * CH:(ci + 1) * CH])
            xt_tiles[ci] = (th, tl)

        load_xt(0)

        def load_whead(pref, hiname, loname, ci, hj):
            """Stream one [128, EC, 128] hi/lo weight pair (head or f tile)."""
            wh = wk.tile([128, EC, 128], f8, name=f"{pref}h{ci}_{hj}",
                         tag="wp", bufs=6)
            nc.sync.dma_start(
                wh[:], io[hiname].rearrange("(a p) n -> p a n", p=128)
                [:, :, hj * 128:(hj + 1) * 128])
            wl = wk.tile([128, EC, 128], f8, name=f"{pref}l{ci}_{hj}",
                         tag="wp", bufs=6)
            nc.sync.dma_start(
                wl[:], io[loname].rearrange("(a p) n -> p a n", p=128)
                [:, :, hj * 128:(hj + 1) * 128])
            return wh, wl

        # ---------------- constants ----------------
        ident = cpool.tile([128, 128], bf)
        nc.sync.dma_start(ident[:], io["ident"][:])
        onec = cpool.tile([128, 1], bf)
        nc.sync.dma_start(onec[:], io["onec"][:])
        masks = cpool.tile([128, 4, CH], bf)
        nc.sync.dma_start(masks[:], io["masks"].rearrange("m p n -> p m n"))
        p64 = cpool.tile([128, 128], bf)
        nc.sync.dma_start(p64[:], io["p64"][:])
        wgh = cpool.tile([GC, HL], f8)
        nc.sync.dma_start(wgh[:], io["wgh"][:])
        wgl = cpool.tile([GC, HL], f8)
        nc.sync.dma_start(wgl[:], io["wgl"][:])
        eps_e = cpool.tile([128, 1], f32)
        nc.vector.memset(eps_e[:], EPS)
        eps_qk = cpool.tile([128, 1], f32)
        nc.vector.memset(eps_qk[:], eps_qk_v)

        kT = big.tile([128, HL, T], bf)           # final K, feature-major
        vtile = big.tile([128, T // 128, HDL], bf)  # final V, token-major
        wot = big.tile([128, HL, E], bf)          # Wo resident
        nc.sync.dma_start(
            wot[:], io["woT"].rearrange("(a p) n -> p a n", p=128))

        cin = dram.tile([T, E], bf)
        cout = dram.tile([T, E], bf)

        scols = []     # per t-tile rmsnorm(x) scale (128,1) f32
        s2cols = {}    # per t-tile s2^2 (128,1) f32

        groups = [[0, 1, 2, 3], [4, 5, 6, 7]]

        def mm_split(ps, whi, wlo, xhi, xlo, ncol=None):
            """ps += (whi+wlo).T @ (xhi+xlo) minus lo*lo, DoubleRow fp8."""
            NPAIR = EC // 2
            for i in range(NPAIR):
                s = slice(2 * i, 2 * i + 2)
                nc.tensor.matmul(ps, whi[:, s, :], xhi[:, s, :], perf_mode=DR,
                                 start=(i == 0), stop=False)
            for i in range(NPAIR):
                s = slice(2 * i, 2 * i + 2)
                nc.tensor.matmul(ps, wlo[:, s, :], xhi[:, s, :], perf_mode=DR,
                                 start=False, stop=False)
            for i in range(NPAIR):
                s = slice(2 * i, 2 * i + 2)
                nc.tensor.matmul(ps, whi[:, s, :], xlo[:, s, :], perf_mode=DR,
                                 start=False, stop=(i == NPAIR - 1))

        def row_stats_sq(x_tt, name):
            """mean of squares per row of a (128, E) bf16 tile -> (128,1)."""
            bnt = colp.tile([128, 4, 6], f32, name=f"{name}_bnt", tag="bnt",
                            bufs=2)
            for i in range(4):
                nc.vector.bn_stats(bnt[:, i, :],
                                   x_tt[:, i * 512:(i + 1) * 512])
            agg = colp.tile([128, 2], f32, name=f"{name}_agg", tag="agg",
                            bufs=2)
            nc.vector.bn_aggr(agg[:], bnt[:])
            m2 = colp.tile([128, 1], f32, name=f"{name}_m2", tag="c1", bufs=8)
            nc.vector.tensor_tensor(m2[:], agg[:, 0:1], agg[:, 0:1],
                                    op=OP.mult)
            msq = colp.tile([128, 1], f32, name=f"{name}_msq", tag="c1",
                            bufs=8)
            nc.vector.tensor_tensor(msq[:], m2[:], agg[:, 1:2], op=OP.add)
            return msq

        def rope_norm(src_ps, c, h, kind, dst, cos2, sin2):
            """dst = rmsnorm(rope(q))*1.2 from scaled psum (scale cancels)."""
            sb = wk.tile([128, CH], bf, name=f"{kind}sb{c}_{h}", tag="qk",
                         bufs=6)
            nc.scalar.copy(sb[:], src_ps[:])
            sq = wk.tile([128, CH], bf, name=f"{kind}sq{c}_{h}", tag="qk",
                         bufs=6)
            nc.scalar.activation(sq[:], src_ps[:], AF.Square)
            ssr = wk.tile([128, CH], f32, name=f"{kind}ss{c}_{h}", tag="ssr",
                          bufs=2)
            nc.gpsimd.partition_all_reduce(ssr[:], sq[:], channels=128,
                                           reduce_op=RED.add)
            lnr = wk.tile([128, CH], bf, name=f"{kind}ln{c}_{h}", tag="rs",
                          bufs=3)
            nc.scalar.activation(lnr[:], ssr[:], AF.Ln, scale=qk_ln_scale,
                                 bias=eps_qk[:])
            rs2 = wk.tile([128, CH], bf, name=f"{kind}rs{c}_{h}", tag="rs",
                          bufs=3)
            nc.scalar.activation(rs2[:], lnr[:], AF.Exp, scale=-0.5)
            swp_ps = psp.tile([128, CH], f32, name=f"{kind}sw{c}_{h}",
                              tag="ps", bufs=8)
            nc.tensor.matmul(swp_ps[:], p64[:], sb[:], start=True, stop=True)
            ta = wk.tile([128, CH], bf, name=f"{kind}ta{c}_{h}", tag="qk",
                         bufs=6)
            tb = wk.tile([128, CH], bf, name=f"{kind}tb{c}_{h}", tag="qk",
                         bufs=6)
            ro = wk.tile([128, CH], bf, name=f"{kind}ro{c}_{h}", tag="qk",
                         bufs=6)
            nc.gpsimd.tensor_tensor(ta[:], sb[:], cos2[:], op=OP.mult)
            nc.vector.tensor_tensor(tb[:], swp_ps[:], sin2[:], op=OP.mult)
            nc.vector.tensor_tensor(ro[:], ta[:], tb[:], op=OP.add)
            nc.vector.tensor_tensor(dst, ro[:], rs2[:], op=OP.mult)

        # =================== per-chunk MLP (pipelined) ===================
        def emit_mlp(cp):
            csl = slice(cp * CH, (cp + 1) * CH)
            # s2 stats from cout (token-major)
            for tt in range(4):
                rows = slice(cp * CH + tt * 128, cp * CH + (tt + 1) * 128)
                x1_tt = wk.tile([128, E], bf, name=f"x1{cp}_{tt}", tag="xq",
                                bufs=2)
                nc.sync.dma_start(x1_tt[:], cout[rows, :])
                msq1 = row_stats_sq(x1_tt, f"s2_{cp}_{tt}")
                ln1 = colp.tile([128, 1], f32, name=f"ln1{cp}_{tt}", tag="c1",
                                bufs=8)
                nc.scalar.activation(ln1[:], msq1[:], AF.Ln, scale=1.0,
                                     bias=eps_e[:])
                s2sq = colp.tile([128, 1], f32, name=f"s2sq{cp}_{tt}",
                                 tag="s2col", bufs=8)
                nc.scalar.activation(s2sq[:], ln1[:], AF.Exp, scale=-1.0)
                s2cols[cp * 4 + tt] = s2sq

            # x1 feature-major + fp8 split (streamed per e-tile)
            x1h = big.tile([128, EC, CH], f8, name=f"x1h{cp}", tag="x1h",
                           bufs=1)
            x1l = big.tile([128, EC, CH], f8, name=f"x1l{cp}", tag="x1l",
                           bufs=1)
            for e in range(EC):
                x1e = wk.tile([128, CH], bf, name=f"x1e{cp}_{e}", tag="x1e",
                              bufs=4)
                nc.sync.dma_start_transpose(
                    x1e[:], cout[csl, e * 128:(e + 1) * 128])
                nc.scalar.activation(x1h[:, e, :], x1e[:], AF.Copy,
                                     scale=SX)
                nc.vector.scalar_tensor_tensor(x1l[:, e, :], x1e[:],
                                               SX, x1h[:, e, :],
                                               op0=OP.mult, op1=OP.subtract)

            # fc: u2[f] = relu(x1 @ Wfc_f)^2  (raw, s2^2 applied post-mp)
            u2s = []
            for f in range(FCT):
                wfh_t, wfl_t = load_whead("wf", "wfh", "wfl", cp, f)
                u_ps = psp.tile([128, CH], f32, name=f"ups{cp}_{f}", tag="ps",
                                bufs=8)
                mm_split(u_ps[:], wfh_t, wfl_t, x1h, x1l)
                ur = wk.tile([128, CH], bf, name=f"ur{cp}_{f}", tag="p",
                             bufs=6)
                nc.scalar.activation(ur[:], u_ps[:], AF.Relu, scale=PSC)
                u2 = wk.tile([128, CH], bf, name=f"u2{cp}_{f}", tag="u2",
                             bufs=FCT + 1)
                nc.gpsimd.tensor_tensor(u2[:], ur[:], ur[:], op=OP.mult)
                u2s.append(u2)

            # mp: out rows += s2^2 * (u2 @ Wmlp_ot)
            for ot in range(4):
                osl = slice(ot * 512, (ot + 1) * 512)
                wm_ot = big.tile([128, FCT, 512], bf, name=f"wm{cp}_{ot}",
                                 tag="wm", bufs=1)
                nc.sync.dma_start(
                    wm_ot[:],
                    io["wmlpT"].rearrange("(a p) n -> p a n", p=128)[:, :, osl])
                for tl in range(4):
                    tsl = slice(tl * 128, (tl + 1) * 128)
                    mp = psp.tile([128, 512], f32, name=f"mp{cp}_{ot}_{tl}",
                                  tag="ps", bufs=8)
                    for f in range(FCT):
                        nc.tensor.matmul(mp[:], u2s[f][:, tsl], wm_ot[:, f, :],
                                         start=(f == 0), stop=(f == FCT - 1))
                    o_sb = wk.tile([128, 512], bf, name=f"o{cp}_{ot}_{tl}",
                                   tag="of", bufs=3)
                    nc.vector.tensor_scalar(o_sb[:], mp[:],
                                            s2cols[cp * 4 + tl][:], None,
                                            op0=OP.mult)
                    rows = slice(cp * CH + tl * 128, cp * CH + (tl + 1) * 128)
                    nc.sync.dma_start(io["out"][rows, osl], o_sb[:])

        # ======================= main chunk loop =======================
        for c in range(nch):
            csl = slice(c * CH, (c + 1) * CH)
            xth, xtl = xt_tiles.pop(c)
            cos2 = wk.tile([128, CH], bf, name=f"cos2_{c}", tag="cs", bufs=4)
            nc.sync.dma_start(cos2[:], io["cos2"][:, csl])
            sin2 = wk.tile([128, CH], bf, name=f"sin2_{c}", tag="cs", bufs=4)
            nc.sync.dma_start(sin2[:], io["sin2"][:, csl])

            # xq t-tiles + scol[t] = rsqrt(mean(x^2)+eps)
            for tt in range(4):
                rows = slice(c * CH + tt * 128, c * CH + (tt + 1) * 128)
                xq_tt = wk.tile([128, E], bf, name=f"xq{c}_{tt}", tag="xq",
                                bufs=2)
                nc.sync.dma_start(xq_tt[:], io["xq"][rows, :])
                msq = row_stats_sq(xq_tt, f"s{c}_{tt}")
                lnm = colp.tile([128, 1], f32, name=f"lnm{c}_{tt}", tag="c1",
                                bufs=8)
                # mean(x^2) = 16*msq  (xq = x/4)
                nc.scalar.activation(lnm[:], msq[:], AF.Ln, scale=16.0,
                                     bias=eps_e[:])
                scol = colp.tile([128, 1], f32, name=f"scol{c}_{tt}",
                                 tag="scol", bufs=4 * nch)
                nc.scalar.activation(scol[:], lnm[:], AF.Exp, scale=-0.5)
                scols.append(scol)

            # ---- gate (token-major): 3-term plain fp8 matmul ----
            g3s = []
            for tt in range(4):
                tsl = slice(tt * 128, (tt + 1) * 128)
                g_ps = psp.tile([128, HL], f32, name=f"gps{c}_{tt}", tag="ps",
                                bufs=8)
                nc.tensor.matmul(g_ps[:], xth[0:GC, 0, tsl], wgh[:],
                                 start=True, stop=False)
                nc.tensor.matmul(g_ps[:], xtl[0:GC, 0, tsl], wgh[:],
                                 start=False, stop=False)
                nc.tensor.matmul(g_ps[:], xth[0:GC, 0, tsl], wgl[:],
                                 start=False, stop=True)
                zs = colp.tile([128, HL], f32, name=f"zs{c}_{tt}", tag="g4",
                               bufs=3)
                nc.vector.tensor_scalar(zs[:], g_ps[:], scols[c * 4 + tt][:],
                                        1.0 / (SG * SX), op0=OP.mult,
                                        op1=OP.mult)
                ge = colp.tile([128, HL], f32, name=f"ge{c}_{tt}", tag="g4",
                               bufs=3)
                nc.scalar.activation(ge[:], zs[:], AF.Exp, scale=-1.0)
                gd = colp.tile([128, HL], f32, name=f"gd{c}_{tt}", tag="g4",
                               bufs=3)
                nc.vector.tensor_scalar(gd[:], ge[:], 1.0, None, op0=OP.add)
                gr = colp.tile([128, HL], f32, name=f"gr{c}_{tt}", tag="g4",
                               bufs=3)
                nc.vector.reciprocal(gr[:], gd[:])
                g3 = colp.tile([128, HL], f32, name=f"g3{c}_{tt}", tag="g3",
                               bufs=4)
                nc.vector.tensor_scalar(g3[:], gr[:], 3.0, None, op0=OP.mult)
                g3s.append(g3)

            # ---- q/k projections + rope + norm ----
            qfs = []
            for h in range(HL):
                wqh_t, wql_t = load_whead("wq", "wqh", "wql", c, h)
                q_ps = psp.tile([128, CH], f32, name=f"qps{c}_{h}", tag="ps",
                                bufs=8)
                mm_split(q_ps[:], wqh_t, wql_t, xth, xtl)
                qf = wk.tile([128, CH], bf, name=f"qf{c}_{h}", tag="qf",
                             bufs=5)
                rope_norm(q_ps, c, h, "q", qf[:], cos2, sin2)
                qfs.append(qf)

                wkh_t, wkl_t = load_whead("wk", "wkh", "wkl", c, h)
                k_ps = psp.tile([128, CH], f32, name=f"kps{c}_{h}", tag="ps",
                                bufs=8)
                mm_split(k_ps[:], wkh_t, wkl_t, xth, xtl)
                rope_norm(k_ps, c, h, "k", kT[:, h, csl], cos2, sin2)

            # ---- v projection (feature-major) + transpose + assembly ----
            vfs = []
            for h in range(HL):
                wvh_t, wvl_t = load_whead("wv", "wvh", "wvl", c, h)
                v_ps = psp.tile([128, CH], f32, name=f"vps{c}_{h}", tag="ps",
                                bufs=8)
                mm_split(v_ps[:], wvh_t, wvl_t, xth, xtl)
                vf = wk.tile([128, CH], bf, name=f"vf{c}_{h}", tag="vf",
                             bufs=5)
                nc.scalar.copy(vf[:], v_ps[:])
                vfs.append(vf)

            for tt in range(4):
                tsl = slice(tt * 128, (tt + 1) * 128)
                vt_ps = psp.tile([128, HDL], f32, name=f"vtps{c}_{tt}",
                                 tag="ps", bufs=8)
                for h in range(HL):
                    nc.tensor.matmul(vt_ps[:, h * D:(h + 1) * D],
                                     vfs[h][:, tsl], ident[:],
                                     start=True, stop=True)
                vem_t = wk.tile([128, HDL], bf, name=f"vem{c}_{tt}",
                                tag="vem", bufs=2)
                rows = slice(c * CH + tt * 128, c * CH + (tt + 1) * 128)
                nc.sync.dma_start(vem_t[:], io["vem"][rows, :])
                gv = wk.tile([128, HDL], bf, name=f"gv{c}_{tt}", tag="gv",
                             bufs=2)
                for h in range(HL):
                    nc.gpsimd.tensor_scalar(
                        gv[:, h * D:(h + 1) * D], vem_t[:, h * D:(h + 1) * D],
                        g3s[tt][:, h:h + 1], None, op0=OP.mult)
                scol_v = colp.tile([128, 1], f32, name=f"sclv{c}_{tt}",
                                   tag="c1", bufs=8)
                nc.vector.tensor_scalar(scol_v[:], scols[c * 4 + tt][:], PSC,
                                        None, op0=OP.mult)
                nc.vector.scalar_tensor_tensor(
                    vtile[:, c * 4 + tt, :], vt_ps[:], scol_v[:],
                    gv[:], op0=OP.mult, op1=OP.add)

            if c + 1 < nch:
                load_xt(c + 1)

            # ---- previous chunk's MLP (hides AllReduce latency) ----
            if c >= 1:
                emit_mlp(c - 1)

            # ---- attention ----
            yTfs = []
            nsb = 4 * (c + 1)
            for h in range(HL):
                sums_ps = psp.tile([1, CH], f32, name=f"sums{c}_{h}",
                                   tag="ps", bufs=8)
                yT_ps = psp.tile([128, CH], f32, name=f"yT{c}_{h}", tag="ps",
                                 bufs=8)
                for sb_i in range(nsb):
                    sc_ps = psp.tile([128, CH], f32, name=f"sc{c}_{h}_{sb_i}",
                                     tag="ps", bufs=8)
                    nc.tensor.matmul(sc_ps[:],
                                     kT[:, h, sb_i * 128:(sb_i + 1) * 128],
                                     qfs[h][:], start=True, stop=True)
                    p0 = wk.tile([128, CH], bf, name=f"p0{c}_{h}_{sb_i}",
                                 tag="p", bufs=6)
                    nc.scalar.activation(p0[:], sc_ps[:], AF.Exp,
                                         scale=inv_sqrt_d)
                    if sb_i >= 4 * c:
                        pm = wk.tile([128, CH], bf, name=f"pm{c}_{h}_{sb_i}",
                                     tag="p", bufs=6)
                        nc.vector.tensor_tensor(pm[:], p0[:],
                                                masks[:, sb_i - 4 * c, :],
                                                op=OP.mult)
                    else:
                        pm = p0
                    nc.tensor.matmul(sums_ps[:], onec[:], pm[:],
                                     start=(sb_i == 0), stop=(sb_i == nsb - 1))
                    nc.tensor.matmul(yT_ps[:],
                                     vtile[:, sb_i, h * D:(h + 1) * D],
                                     pm[:], start=(sb_i == 0),
                                     stop=(sb_i == nsb - 1))
                isr = colp.tile([1, CH], bf, name=f"isr{c}_{h}", tag="r512b",
                                bufs=2)
                with nc.allow_low_precision(reason="softmax 1/sum in bf16"):
                    nc.vector.reciprocal(isr[:], sums_ps[:])
                ib = wk.tile([128, CH], bf, name=f"ib{c}_{h}", tag="ib",
                             bufs=2)
                nc.gpsimd.partition_broadcast(ib[:], isr[0:1, :])
                yTf = wk.tile([128, CH], bf, name=f"yTf{c}_{h}", tag="y",
                              bufs=5)
                nc.vector.tensor_tensor(yTf[:], yT_ps[:], ib[:], op=OP.mult)
                yTfs.append(yTf)

            # ---- Wo partial + 0.25*x, straight to AR bounce ----
            for tt in range(4):
                tsl = slice(tt * 128, (tt + 1) * 128)
                rows = slice(c * CH + tt * 128, c * CH + (tt + 1) * 128)
                xqw = wk.tile([128, E], bf, name=f"xqw{c}_{tt}", tag="xq",
                              bufs=2)
                nc.sync.dma_start(xqw[:], io["xq"][rows, :])
                for ot in range(4):
                    osl = slice(ot * 512, (ot + 1) * 512)
                    wo_ps = psp.tile([128, 512], f32,
                                     name=f"wops{c}_{tt}_{ot}", tag="ps",
                                     bufs=8)
                    for h in range(HL):
                        nc.tensor.matmul(wo_ps[:], yTfs[h][:, tsl],
                                         wot[:, h, osl], start=(h == 0),
                                         stop=(h == HL - 1))
                    aro = wk.tile([128, 512], bf, name=f"aro{c}_{tt}_{ot}",
                                  tag="p", bufs=6)
                    nc.vector.tensor_tensor(aro[:], wo_ps[:], xqw[:, osl],
                                            op=OP.add)
                    nc.sync.dma_start(cin[rows, osl], aro[:])

            # ---- AllReduce this chunk within the batch group ----
            if NOAR:
                nc.sync.dma_start(cout[csl, :], cin[csl, :])
            else:
                nc.gpsimd.collective_compute(
                    "AllReduce", mybir.AluOpType.add, replica_groups=groups,
                    ins=[cin[csl, :].opt()], outs=[cout[csl, :].opt()])
            nc.sync.dma_start(io["out_x1"][csl, :], cout[csl, :])

        emit_mlp(nch - 1)


def _pin_act_tables():
    """Force every activation onto natural_log_exp_and_others so the table
    is loaded once instead of thrashing between per-function sets."""
    import concourse.bacc as bacc_mod
    import concourse.mybir as mybir
    if getattr(bacc_mod, "_act_tables_pinned", False):
        return
    AF = mybir.ActivationFunctionType
    mine = {AF.Exp, AF.Ln, AF.Square, AF.Relu, AF.Copy, AF.Identity}
    orig = bacc_mod.get_activation_tables

    def patched(arch):
        t = orig(arch)
        out = {}
        for name, funcs in t.items():
            if name == "natural_log_exp_and_others":
                out[name] = set(funcs)
            else:
                out[name] = set(funcs) - mine
        return out

    bacc_mod.get_activation_tables = patched
    bacc_mod._act_tables_pinned = True


def build_nc(T=T_FULL, num_devices=NCORES):
    from concourse import bacc
    import concourse.tile as tile
    _pin_act_tables()
    nc = bacc.Bacc("TRN2", target_bir_lowering=False, debug=False,
                   enable_asserts=True, num_devices=num_devices)
    io = declare_io(nc, T)
    with tile.TileContext(nc) as tc:
        emit(tc, io, T)
    nc.compile()
    return nc


def combine_outputs(results, T=T_FULL):
    out = np.zeros((B, T, E), np.float32)
    for c in range(NCORES):
        out[c // 4] += np.asarray(results[c]["out"]).astype(np.float32)
    for b in range(B):
        out[b] += np.asarray(results[b * 4]["out_x1"]).astype(np.float32)
    return out


def kernel(**inputs):
    from concourse.bass_utils import run_bass_kernel_spmd
    in_maps = shard_inputs(**inputs)
    nc = build_nc(T_FULL)
    res = run_bass_kernel_spmd(nc, in_maps, core_ids=list(range(NCORES)))
    return combine_outputs(res.results, T_FULL)
